# revision 10
# baseline (speedup 1.0000x reference)
"""Trainium2 Bass kernel for nn_DivEncLayer (grouped per-slice MLP 8->32->1).

Reference computation (per batch row b, per slice q of 128):
    xs = x.reshape(B, 128, 8)
    h  = ELU(xs[b,q,:] @ W1[q] + b1[q])            # (32,)
    h  = (h - mov_mean[q]) * gamma[q]/sqrt(mov_var[q]+eps) + beta[q]
    out[b,q] = h @ W2[q] + b2[q]

v2 strategy (pure data parallel over 8 NeuronCores, B=32768 -> 4096/core):
  * HOST pre-transposes x -> xT [1024, bc] (bf16 by default): the device
    DMAs c-major tiles directly; no PE transposes, no transpose drains.
  * BN affine + W2 fold into w2p[q,h] on host; final bias bfin[q] added
    on host (device output is the pure matmul part, laid out [p, b] with
    p a fixed permutation of q).
  * ELU(u) = ReLU(u) + min(exp(u),1) - 1 (exact identity); the exp part
    uses the Schraudolph int16 bitcast trick (see baseline docstring).
  * dense1 is ROW-TILED on the PE: per c-group g (128 c = 16 slices),
    4 concurrent matmuls at tile_position=(32r, 0), each K=32 (4 slices
    x 8 c), M=128 (4 slices x 32 h), N=512 batch.  u_r lands in PSUM
    bank r (4 banks, single-buffered; the drains free them for the next
    group).
  * Schraudolph drain i = int16(A*u + b0): split between ACT (activation
    Identity, bias/scale) and DVE (tensor_scalar mult/add) by the DSPLIT
    knob; T = min(i,0)+c and R = max(i,0)/A on DVE at 16-bit rate
    (optionally GPSIMD via GPST/GPSR knobs).
  * dense2 is COL-TILED: per group g and band c, matmul with
    tile_position=(0, 32c): lhsT = zero-padded [128, 32] tile holding
    w2p of slices (g, 4c+j) in columns 4g+j; rhs = mid tile r=c.  All
    16 chain members (8 groups x {R,T}) of band c accumulate into
    o[32c:32c+32]; dead columns add exact zeros, so the shared
    accumulation chain stays correct.  o partition p = 32c + 4g + j
    holds slice q = 16g + 4c + j (host unpermutes).
  * o [128, 512] f32 -> drain -> DMA out [p, b]; host adds bfin and
    transposes to [b, q].

Known walrus/HW constraints handled here:
  * any instruction encoding supports only ONE semaphore wait -> _split_waits
  * PSUM accumulation chains must share one tile_position
  * col-tiled matmul PSUM output base partition must be 32-aligned
"""

import sys

for _p in ("/opt/trn_rl_repo", "/root/.axon_site/_ro/trn_rl_repo"):
    if _p not in sys.path:
        sys.path.append(_p)

import contextlib
import os as _os

import numpy as np

import concourse.bass as bass
import concourse.tile as tile
from concourse import mybir
from concourse.bass_utils import run_bass_kernel_spmd

F32 = mybir.dt.float32
F32R = mybir.dt.float32r
BF16 = mybir.dt.bfloat16
INT16 = mybir.dt.int16

Q, S, H = 128, 8, 32
C = Q * S                      # 1024
NCORES = 8
BN_EPS = 1e-3

NB = 512                       # batch tile (matmul free dim)
NG = 8                         # c/slice groups of 16 slices (128 partitions)

MID_DT = BF16

# Knobs:
#   DPAT: per-group drain engine pattern: 'A' = ACT drains both u halves,
#         'S' = split (ACT half 0, DVE half 1), 'D' = DVE both
#   GPST/GPSR: per-group flags 'G'/'D' -> T (resp. R) op on GPSIMD or DVE
#   OUTQ: engine for the o PSUM->SBUF drain ('A' or 'D')
#   ABLATE: comma list of stages to skip (timing experiments only):
#           nodma,nod1,nodrain,notr,nod2
DPAT = _os.environ.get("DPAT", "ASASASAS")
GPST = _os.environ.get("GPST", "DDDDDDDD")
GPSR = _os.environ.get("GPSR", "DDDDDDDD")
OUTQ_ENG = _os.environ.get("OUTQ", "D")
XDT_NAME = _os.environ.get("XDT", "bf16")
XDT = {"bf16": BF16, "f32r": F32R}[XDT_NAME]
ABLATE = set(filter(None, _os.environ.get("ABLATE", "").split(",")))
# ablations are cumulative back-to-front: skipping a producer skips all
# downstream consumers too
if "nod1" in ABLATE:
    ABLATE.add("nodrain")
if "nodrain" in ABLATE:
    ABLATE.add("notr")
if "notr" in ABLATE:
    ABLATE.add("nod2")
if "nod2" in ABLATE:
    ABLATE.add("noout")

# Schraudolph int16-exp constants (bf16 bit format):
#   i  = round-ish(SCHRA_A*u + SCHRA_B0)          (drain, int16)
#   T  = bitcast_bf16(min(i,0) + (SCHRA_BE - SCHRA_C)) ~= min(e^u, 1)
#   R  = max(i,0) * (1/SCHRA_A)                   ~= relu(u)
_MANT = 128.0
SCHRA_A = _MANT / float(np.log(2.0))          # 184.664
SCHRA_BE = 127 * 128
SCHRA_B0 = float(_os.environ.get("SCHRA_B0", "1.25"))
SCHRA_C = float(_os.environ.get("SCHRA_C", "4"))

_NOPN = [0]


def _split_waits(tc):
    """walrus supports only one sync-wait command per instruction; Tile can
    emit several.  Precede every multi-wait instruction with same-engine
    NoOps carrying all but the last wait."""
    orig = tc._add_instruction

    def patched(inst):
        si = inst.sync_info
        if (
            not inst.name.startswith("waitnop")
            and si is not None
            and len(si.on_wait) > 1
        ):
            for w in si.on_wait[:-1]:
                _NOPN[0] += 1
                nop = mybir.InstNoOp(name=f"waitnop-{_NOPN[0]}", ins=[], outs=[])
                nop.engine = inst.engine
                nop.sync_info = mybir.SyncInfo(on_wait=[w], on_update=[])
                orig(nop)
            inst.sync_info = mybir.SyncInfo(
                on_wait=[si.on_wait[-1]], on_update=list(si.on_update)
            )
        return orig(inst)

    tc._add_instruction = patched

    def patched_dab(tick_clock, wait_clock):
        from concourse.vector_clock import ScopedClock

        nc = tc.nc
        drain_inst = nc.sync.drain()
        wait_clock.add_sem_waits(
            drain_inst.ins, ScopedClock({None: tick_clock.global_clock})
        )
        si = drain_inst.ins.sync_info
        if si is not None and len(si.on_wait) > 1:
            extra = list(si.on_wait[1:])
            drain_inst.ins.sync_info = mybir.SyncInfo(
                on_wait=[si.on_wait[0]], on_update=list(si.on_update)
            )
            for w in extra:
                n = nc.sync.nop(nofuse=True)
                n.ins.sync_info = mybir.SyncInfo(on_wait=[w], on_update=[])

        nc.all_engine_barrier()
        assert tc.sems is not None
        popped = nc._tile_sem_poison_stack.pop()
        assert popped is tc._sem_poison
        nc.clear_and_free_semaphores(list(tc.sems.allocated().values()))
        nc.all_engine_barrier()

    tc._drain_and_barrier = patched_dab


def _host_pack(W1, b1, gamma, beta, mov_mean, mov_var, W2, b2):
    """Fold BN into second dense; pack block weights for the PE layouts."""
    import ml_dtypes

    W1 = np.asarray(W1, np.float32).reshape(Q, S, H)
    b1 = np.asarray(b1, np.float32).reshape(Q, H)
    gamma = np.asarray(gamma, np.float32).reshape(Q, H)
    beta = np.asarray(beta, np.float32).reshape(Q, H)
    mean = np.asarray(mov_mean, np.float32).reshape(Q, H)
    var = np.asarray(mov_var, np.float32).reshape(Q, H)
    W2 = np.asarray(W2, np.float32).reshape(Q, H)
    b2 = np.asarray(b2, np.float32).reshape(Q)
    assert not np.any(b1 != 0.0), "Schraudolph path requires b1 == 0"

    inv = gamma / np.sqrt(var + BN_EPS)
    w2p = (inv * W2).astype(np.float32)                      # [Q,H]
    # out = sum_h w2p*(ReLU(u) + min(e^u,1)) + bfin
    bfin = (b2 + ((beta - mean * inv) * W2).sum(-1) - w2p.sum(-1)).astype(np.float32)

    # dense1 row-tile stationaries: w1sb[32r + (8j + s), g, 32j + h]
    #   = W1[q = 16g + 4r + j, s, h]
    w1sb = np.zeros((128, NG, 128), np.float32)
    for g in range(NG):
        for r in range(4):
            for j in range(4):
                q = 16 * g + 4 * r + j
                w1sb[32 * r + 8 * j:32 * r + 8 * j + 8, g, 32 * j:32 * j + 32] = W1[q]

    # dense2 col-tile stationaries (zero-padded per (g, c)):
    #   w2sb[32j + h, g, c, 4g + j] = w2p[q = 16g + 4c + j, h]
    w2sb = np.zeros((128, NG, 4, 32), np.float32)
    for g in range(NG):
        for c in range(4):
            for j in range(4):
                q = 16 * g + 4 * c + j
                w2sb[32 * j:32 * j + 32, g, c, 4 * g + j] = w2p[q]

    if XDT == BF16:
        w1sb = w1sb.astype(ml_dtypes.bfloat16)
    w2sb = w2sb.astype(ml_dtypes.bfloat16)

    # output partition permutation: p = 32c + 4g + j  <->  q = 16g + 4c + j
    perm = np.zeros(128, np.int64)
    for g in range(NG):
        for c in range(4):
            for j in range(4):
                perm[32 * c + 4 * g + j] = 16 * g + 4 * c + j
    return w1sb, w2sb, bfin, perm


def _build(bc, rep=1, inner=1):
    """Build the Bass program for one core processing bc batch rows."""
    nc = bass.Bass()

    xt_d = nc.dram_tensor("xt", [C, bc], XDT, kind="ExternalInput")
    w1_d = nc.dram_tensor("w1sb", [128, NG, 128], XDT, kind="ExternalInput")
    w2_d = nc.dram_tensor("w2sb", [128, NG, 4, 32], MID_DT, kind="ExternalInput")
    # sc[:, 0] = b0 (drain bias), sc[:, 1] = 16256 - c (T-op addend)
    sc_d = nc.dram_tensor("schra", [128, 2], F32, kind="ExternalInput")
    # output laid out [p, b] -- host unpermutes p->q, transposes, adds bfin
    out_d = nc.dram_tensor("out", [128, bc], F32, kind="ExternalOutput")

    n_tiles = bc // NB
    Ident = mybir.ActivationFunctionType.Identity
    Copy = mybir.ActivationFunctionType.Copy
    Relu = mybir.ActivationFunctionType.Relu
    Add = mybir.AluOpType.add
    Max = mybir.AluOpType.max
    Min = mybir.AluOpType.min
    Mult = mybir.AluOpType.mult

    with tile.TileContext(nc) as tc:
        _split_waits(tc)
        with (
            tc.tile_pool(name="singles", bufs=1) as singles,
            tc.tile_pool(name="xt", bufs=3) as xt_pool,
            tc.tile_pool(name="iw", bufs=2) as iw_pool,
            tc.tile_pool(name="rt", bufs=2) as rt_pool,
            tc.tile_pool(name="outq", bufs=2) as outq_pool,
            tc.tile_pool(name="ps_u", bufs=3, space="PSUM") as ps_u,
            tc.tile_pool(name="ps_o", bufs=2, space="PSUM") as ps_o,
        ):
            w1t = singles.tile([128, NG, 128], XDT)
            w2t = singles.tile([128, NG, 4, 32], MID_DT)
            schra = singles.tile([128, 2], F32)
            zbias = singles.tile([128, 1], F32)
            wdum = singles.tile([128, 8], F32)

            nc.sync.dma_start(w1t[:], w1_d[:])
            nc.sync.dma_start(w2t[:], w2_d[:])
            nc.sync.dma_start(schra[:], sc_d[:])
            nc.gpsimd.memset(zbias[:], 0.0)
            xt_fixed = None
            if "nodma" in ABLATE:
                xt_fixed = singles.tile([128, NB], XDT)
                nc.sync.dma_start(xt_fixed[:], xt_d[0:128, 0:NB])

            # Warmup: make each engine observe each one-time producer once so
            # steady-state instructions need at most one semaphore wait.
            nc.scalar.activation(wdum[:, 1:2], schra[:, 0:1], Relu)
            nc.vector.tensor_scalar_add(wdum[:, 2:3], zbias[:], schra[:, 1:2])
            nc.vector.tensor_scalar_max(wdum[:, 3:4], schra[:, 0:1], 0.0)
            nc.gpsimd.tensor_scalar_max(wdum[:, 4:5], schra[:, 1:2], 0.0)
            nc.scalar.activation(wdum[:, 5:6], zbias[:], Relu)

            loop_cm = tc.For_i(0, rep, 1) if rep > 1 else contextlib.nullcontext()
            with loop_cm:
             for _inner in range(inner):
              for n in range(n_tiles):
                o = None if "nod2" in ABLATE else ps_o.tile([128, NB], F32, tag="o")
                mids = {}

                for g in range(NG):
                    # ---- load xT c-group tile [128c, 512b] (pre-transposed)
                    if "nodma" not in ABLATE:
                        xt = xt_pool.tile([128, NB], XDT, tag="xt")
                        nc.sync.dma_start(
                            xt[:], xt_d[128 * g:128 * (g + 1), NB * n:NB * (n + 1)]
                        )
                    else:
                        xt = xt_fixed

                    # ---- dense1: 4 row-tiled concurrent matmuls; u in two
                    # 2-bank halves so drains of one half overlap the next
                    # group's matmuls into the other (pool bufs=3 -> 6 banks)
                    ua = ps_u.tile([128, 2, NB], F32, tag="u")
                    ub = ps_u.tile([128, 2, NB], F32, tag="u")
                    if "nod1" not in ABLATE:
                        for r in range(4):
                            uh = ua if r < 2 else ub
                            nc.tensor.matmul(
                                uh[:, r % 2, :],
                                w1t[32 * r:32 * (r + 1), g, :],
                                xt[32 * r:32 * (r + 1), :],
                                start=True,
                                stop=True,
                                tile_position=(32 * r, 0),
                            )

                    # ---- Schraudolph drain i = int16(A*u + b0), per DPAT
                    iw = iw_pool.tile([128, 4, NB], INT16, tag="I")
                    if "nodrain" not in ABLATE:
                        for hf, uh in ((0, ua), (1, ub)):
                            sl = slice(2 * hf, 2 * hf + 2)
                            on_act = DPAT[g] == "A" or (DPAT[g] == "S" and hf == 0)
                            if on_act:
                                nc.scalar.activation(
                                    iw[:, sl, :], uh[:],
                                    Ident, bias=schra[:, 0:1], scale=SCHRA_A)
                            else:
                                nc.vector.tensor_scalar(
                                    iw[:, sl, :], uh[:],
                                    scalar1=float(SCHRA_A),
                                    scalar2=schra[:, 0:1],
                                    op0=Mult, op1=Add)

                    # ---- T = bitcast(min(i,0) + (16256-c)); R = max(i,0)/A
                    tw = rt_pool.tile([128, 4, NB], INT16, tag="T")
                    rw = rt_pool.tile([128, 4, NB], MID_DT, tag="R")
                    if "notr" not in ABLATE:
                        t_eng = nc.gpsimd if GPST[g] == "G" else nc.vector
                        r_eng = nc.gpsimd if GPSR[g] == "G" else nc.vector
                        t_eng.tensor_scalar(
                            tw[:], iw[:], scalar1=0.0,
                            scalar2=schra[:, 1:2],
                            op0=Min, op1=Add)
                        r_eng.tensor_scalar(
                            rw[:], iw[:], scalar1=0.0,
                            scalar2=float(1.0 / SCHRA_A),
                            op0=Max, op1=Mult)
                    mids[g] = (rw, tw)

                    # ---- dense2 deferred by one group for pipelining
                    if g >= 1 and "nod2" not in ABLATE:
                        _dense2(nc, o, w2t, mids, g - 1)
                if "nod2" not in ABLATE:
                    _dense2(nc, o, w2t, mids, NG - 1, last=True)
                else:
                    mids.clear()

                # ---- drain o PSUM->SBUF and store [p, b] slice
                if "noout" not in ABLATE:
                    outq = outq_pool.tile([128, NB], F32, tag="outq")
                    if OUTQ_ENG == "A":
                        nc.scalar.activation(outq[:], o[:], Copy)
                    else:
                        nc.vector.tensor_copy(outq[:], o[:])
                    nc.sync.dma_start(out_d[:, NB * n:NB * (n + 1)], outq[:])

    return nc


def _dense2(nc, o, w2t, mids, g, last=False):
    """Col-tiled dense2 for group g: 8 matmuls (4 bands x {R, T}) into o."""
    rw, tw = mids[g]
    for c in range(4):
        for si, mid in enumerate((rw, tw)):
            rhs = mid[:, c, :]
            if rhs.dtype == INT16:
                rhs = rhs.bitcast(MID_DT)
            nc.tensor.matmul(
                o[32 * c:32 * (c + 1), :],
                w2t[:, g, c, :],
                rhs,
                start=(g == 0 and si == 0),
                stop=(last and si == 1),
                tile_position=(0, 32 * c),
            )
    del mids[g]


_CACHE = {}


def _get_nc(bc, rep=1, inner=1):
    key = (bc, rep, inner)
    if key not in _CACHE:
        _CACHE[key] = _build(bc, rep, inner)
    return _CACHE[key]


def kernel(x, W1, b1, gamma, beta, mov_mean, mov_var, W2, b2, _rep=1, _inner=1):
    import ml_dtypes

    x = np.asarray(x, np.float32).reshape(-1, C)
    B = x.shape[0]
    w1sb, w2sb, bfin, perm = _host_pack(
        W1, b1, gamma, beta, mov_mean, mov_var, W2, b2
    )

    bc = B // NCORES
    nc = _get_nc(bc, _rep, _inner)

    xT = np.ascontiguousarray(
        x.T.astype(ml_dtypes.bfloat16 if XDT == BF16 else np.float32)
    )  # [C, B]

    schra = np.broadcast_to(
        np.array([SCHRA_B0, SCHRA_BE - SCHRA_C], np.float32), (128, 2)
    ).copy()
    in_maps = [
        {
            "xt": np.ascontiguousarray(xT[:, i * bc:(i + 1) * bc]),
            "w1sb": w1sb,
            "w2sb": w2sb,
            "schra": schra,
        }
        for i in range(NCORES)
    ]
    res = run_bass_kernel_spmd(nc, in_maps, list(range(NCORES)))
    kernel._last_results = res
    # device output is [p, bc] per core; unpermute, transpose + bias on host
    out = np.concatenate(
        [res.results[i]["out"][perm.argsort()].T for i in range(NCORES)], axis=0
    ) + bfin[None, :]
    return np.ascontiguousarray(out, dtype=np.float32)


# revision 14
# speedup vs baseline: 2.4825x; 2.4825x over previous
"""Trainium2 Bass kernel for nn_DivEncLayer (grouped per-slice MLP 8->32->1).

Reference computation (per batch row b, per slice q of 128):
    xs = x.reshape(B, 128, 8)
    h  = ELU(xs[b,q,:] @ W1[q] + b1[q])            # (32,)
    h  = (h - mov_mean[q]) * gamma[q]/sqrt(mov_var[q]+eps) + beta[q]
    out[b,q] = h @ W2[q] + b2[q]

v2 strategy (pure data parallel over 8 NeuronCores, B=32768 -> 4096/core):
  * HOST pre-transposes x -> xT [1024, bc] (bf16 by default): the device
    DMAs c-major tiles directly; no PE transposes, no transpose drains.
  * BN affine + W2 fold into w2p[q,h] on host; final bias bfin[q] added
    on host (device output is the pure matmul part, laid out [p, b] with
    p a fixed permutation of q).
  * ELU(u) = ReLU(u) + min(exp(u),1) - 1 (exact identity); the exp part
    uses the Schraudolph int16 bitcast trick (see baseline docstring).
  * dense1 is ROW-TILED on the PE: per c-group g (128 c = 16 slices),
    4 concurrent matmuls at tile_position=(32r, 0), each K=32 (4 slices
    x 8 c), M=128 (4 slices x 32 h), N=512 batch.  u_r lands in PSUM
    bank r (4 banks, single-buffered; the drains free them for the next
    group).
  * Schraudolph drain i = int16(A*u + b0): split between ACT (activation
    Identity, bias/scale) and DVE (tensor_scalar mult/add) by the DSPLIT
    knob; T = min(i,0)+c and R = max(i,0)/A on DVE at 16-bit rate
    (optionally GPSIMD via GPST/GPSR knobs).
  * dense2 is COL-TILED: per group g and band c, matmul with
    tile_position=(0, 32c): lhsT = zero-padded [128, 32] tile holding
    w2p of slices (g, 4c+j) in columns 4g+j; rhs = mid tile r=c.  All
    16 chain members (8 groups x {R,T}) of band c accumulate into
    o[32c:32c+32]; dead columns add exact zeros, so the shared
    accumulation chain stays correct.  o partition p = 32c + 4g + j
    holds slice q = 16g + 4c + j (host unpermutes).
  * o [128, 512] f32 -> drain -> DMA out [p, b]; host adds bfin and
    transposes to [b, q].

Known walrus/HW constraints handled here:
  * any instruction encoding supports only ONE semaphore wait -> _split_waits
  * PSUM accumulation chains must share one tile_position
  * col-tiled matmul PSUM output base partition must be 32-aligned
"""

import sys

for _p in ("/opt/trn_rl_repo", "/root/.axon_site/_ro/trn_rl_repo"):
    if _p not in sys.path:
        sys.path.append(_p)

import contextlib
import os as _os

import numpy as np

import concourse.bass as bass
import concourse.tile as tile
from concourse import mybir
from concourse.bass_utils import run_bass_kernel_spmd

F32 = mybir.dt.float32
F32R = mybir.dt.float32r
BF16 = mybir.dt.bfloat16
INT16 = mybir.dt.int16

Q, S, H = 128, 8, 32
C = Q * S                      # 1024
NCORES = 8
BN_EPS = 1e-3

NB = 512                       # batch tile (matmul free dim)
NG = 8                         # c/slice groups of 16 slices (128 partitions)

MID_DT = BF16

# Knobs:
#   DPAT: per-group drain engine pattern: 'A' = ACT drains both u halves,
#         'S' = split (ACT half 0, DVE half 1), 'D' = DVE both
#   GPST/GPSR: per-group flags 'G'/'D' -> T (resp. R) op on GPSIMD or DVE
#   OUTQ: engine for the o PSUM->SBUF drain ('A' or 'D')
#   ABLATE: comma list of stages to skip (timing experiments only):
#           nodma,nod1,nodrain,notr,nod2
DPAT = _os.environ.get("DPAT", "ASASASAS")
GPST = _os.environ.get("GPST", "DDDDDDDD")
GPSR = _os.environ.get("GPSR", "DDDDDDDD")
OUTQ_ENG = _os.environ.get("OUTQ", "D")
XDT_NAME = _os.environ.get("XDT", "bf16")
XDT = {"bf16": BF16, "f32r": F32R}[XDT_NAME]
ABLATE = set(filter(None, _os.environ.get("ABLATE", "").split(",")))
# ablations are cumulative back-to-front: skipping a producer skips all
# downstream consumers too
if "nod1" in ABLATE:
    ABLATE.add("nodrain")
if "nodrain" in ABLATE:
    ABLATE.add("notr")
if "notr" in ABLATE:
    ABLATE.add("nod2")
if "nod2" in ABLATE:
    ABLATE.add("noout")

# Schraudolph int16-exp constants (bf16 bit format):
#   i  = round-ish(SCHRA_A*u + SCHRA_B0)          (drain, int16)
#   T  = bitcast_bf16(min(i,0) + (SCHRA_BE - SCHRA_C)) ~= min(e^u, 1)
#   R  = max(i,0) * (1/SCHRA_A)                   ~= relu(u)
_MANT = 128.0
SCHRA_A = _MANT / float(np.log(2.0))          # 184.664
SCHRA_BE = 127 * 128
SCHRA_B0 = float(_os.environ.get("SCHRA_B0", "1.25"))
SCHRA_C = float(_os.environ.get("SCHRA_C", "4"))

_NOPN = [0]


def _split_waits(tc):
    """walrus supports only one sync-wait command per instruction; Tile can
    emit several.  Precede every multi-wait instruction with same-engine
    NoOps carrying all but the last wait."""
    orig = tc._add_instruction

    def patched(inst):
        si = inst.sync_info
        if (
            not inst.name.startswith("waitnop")
            and si is not None
            and len(si.on_wait) > 1
        ):
            for w in si.on_wait[:-1]:
                _NOPN[0] += 1
                nop = mybir.InstNoOp(name=f"waitnop-{_NOPN[0]}", ins=[], outs=[])
                nop.engine = inst.engine
                nop.sync_info = mybir.SyncInfo(on_wait=[w], on_update=[])
                orig(nop)
            inst.sync_info = mybir.SyncInfo(
                on_wait=[si.on_wait[-1]], on_update=list(si.on_update)
            )
        return orig(inst)

    tc._add_instruction = patched

    def patched_dab(tick_clock, wait_clock):
        from concourse.vector_clock import ScopedClock

        nc = tc.nc
        drain_inst = nc.sync.drain()
        wait_clock.add_sem_waits(
            drain_inst.ins, ScopedClock({None: tick_clock.global_clock})
        )
        si = drain_inst.ins.sync_info
        if si is not None and len(si.on_wait) > 1:
            extra = list(si.on_wait[1:])
            drain_inst.ins.sync_info = mybir.SyncInfo(
                on_wait=[si.on_wait[0]], on_update=list(si.on_update)
            )
            for w in extra:
                n = nc.sync.nop(nofuse=True)
                n.ins.sync_info = mybir.SyncInfo(on_wait=[w], on_update=[])

        nc.all_engine_barrier()
        assert tc.sems is not None
        popped = nc._tile_sem_poison_stack.pop()
        assert popped is tc._sem_poison
        nc.clear_and_free_semaphores(list(tc.sems.allocated().values()))
        nc.all_engine_barrier()

    tc._drain_and_barrier = patched_dab


def _host_pack(W1, b1, gamma, beta, mov_mean, mov_var, W2, b2):
    """Fold BN into second dense; pack block weights for the PE layouts."""
    import ml_dtypes

    W1 = np.asarray(W1, np.float32).reshape(Q, S, H)
    b1 = np.asarray(b1, np.float32).reshape(Q, H)
    gamma = np.asarray(gamma, np.float32).reshape(Q, H)
    beta = np.asarray(beta, np.float32).reshape(Q, H)
    mean = np.asarray(mov_mean, np.float32).reshape(Q, H)
    var = np.asarray(mov_var, np.float32).reshape(Q, H)
    W2 = np.asarray(W2, np.float32).reshape(Q, H)
    b2 = np.asarray(b2, np.float32).reshape(Q)
    assert not np.any(b1 != 0.0), "Schraudolph path requires b1 == 0"

    inv = gamma / np.sqrt(var + BN_EPS)
    w2p = (inv * W2).astype(np.float32)                      # [Q,H]
    # out = sum_h w2p*(ReLU(u) + min(e^u,1)) + bfin
    bfin = (b2 + ((beta - mean * inv) * W2).sum(-1) - w2p.sum(-1)).astype(np.float32)

    # dense1 row-tile stationaries: w1sb[32r + (8j + s), g, 32j + h]
    #   = W1[q = 16g + 4r + j, s, h]
    w1sb = np.zeros((128, NG, 128), np.float32)
    for g in range(NG):
        for r in range(4):
            for j in range(4):
                q = 16 * g + 4 * r + j
                w1sb[32 * r + 8 * j:32 * r + 8 * j + 8, g, 32 * j:32 * j + 32] = W1[q]

    # dense2 col-tile stationaries (zero-padded per (g, c)):
    #   w2sb[32j + h, g, c, 4g + j] = w2p[q = 16g + 4c + j, h]
    w2sb = np.zeros((128, NG, 4, 32), np.float32)
    for g in range(NG):
        for c in range(4):
            for j in range(4):
                q = 16 * g + 4 * c + j
                w2sb[32 * j:32 * j + 32, g, c, 4 * g + j] = w2p[q]

    if XDT == BF16:
        w1sb = w1sb.astype(ml_dtypes.bfloat16)
    w2sb = w2sb.astype(ml_dtypes.bfloat16)

    # output partition permutation: p = 32c + 4g + j  <->  q = 16g + 4c + j
    perm = np.zeros(128, np.int64)
    for g in range(NG):
        for c in range(4):
            for j in range(4):
                perm[32 * c + 4 * g + j] = 16 * g + 4 * c + j
    return w1sb, w2sb, bfin, perm


def _build(bc, rep=1, inner=1):
    """Build the Bass program for one core processing bc batch rows."""
    nc = bass.Bass()

    xt_d = nc.dram_tensor("xt", [NG, 128, bc], XDT, kind="ExternalInput")
    w1_d = nc.dram_tensor("w1sb", [128, NG, 128], XDT, kind="ExternalInput")
    w2_d = nc.dram_tensor("w2sb", [128, NG, 4, 32], MID_DT, kind="ExternalInput")
    # sc[:, 0] = b0 (drain bias), sc[:, 1] = 16256 - c (T-op addend)
    sc_d = nc.dram_tensor("schra", [128, 2], F32, kind="ExternalInput")
    # output laid out [p, b] -- host unpermutes p->q, transposes, adds bfin
    out_d = nc.dram_tensor("out", [128, bc], F32, kind="ExternalOutput")

    n_tiles = bc // NB
    Ident = mybir.ActivationFunctionType.Identity
    Copy = mybir.ActivationFunctionType.Copy
    Relu = mybir.ActivationFunctionType.Relu
    Add = mybir.AluOpType.add
    Max = mybir.AluOpType.max
    Min = mybir.AluOpType.min
    Mult = mybir.AluOpType.mult

    with tile.TileContext(nc) as tc:
        _split_waits(tc)
        with (
            tc.tile_pool(name="singles", bufs=1) as singles,
            tc.tile_pool(name="xt", bufs=3) as xt_pool,
            tc.tile_pool(name="iw", bufs=2) as iw_pool,
            tc.tile_pool(name="rt", bufs=2) as rt_pool,
            tc.tile_pool(name="outq", bufs=2) as outq_pool,
            tc.tile_pool(name="ps_u", bufs=3, space="PSUM") as ps_u,
            tc.tile_pool(name="ps_o", bufs=2, space="PSUM") as ps_o,
        ):
            w1t = singles.tile([128, NG, 128], XDT)
            w2t = singles.tile([128, NG, 4, 32], MID_DT)
            schra = singles.tile([128, 2], F32)
            zbias = singles.tile([128, 1], F32)
            wdum = singles.tile([128, 8], F32)

            nc.sync.dma_start(w1t[:], w1_d[:])
            nc.sync.dma_start(w2t[:], w2_d[:])
            nc.sync.dma_start(schra[:], sc_d[:])
            nc.gpsimd.memset(zbias[:], 0.0)
            xt_fixed = None
            if "nodma" in ABLATE:
                xt_fixed = singles.tile([128, NG, NB], XDT)
                nc.sync.dma_start(
                    xt_fixed[:], xt_d[:, :, 0:NB].transpose([1, 0, 2])
                )

            # Warmup: make each engine observe each one-time producer once so
            # steady-state instructions need at most one semaphore wait.
            nc.scalar.activation(wdum[:, 1:2], schra[:, 0:1], Relu)
            nc.vector.tensor_scalar_add(wdum[:, 2:3], zbias[:], schra[:, 1:2])
            nc.vector.tensor_scalar_max(wdum[:, 3:4], schra[:, 0:1], 0.0)
            nc.gpsimd.tensor_scalar_max(wdum[:, 4:5], schra[:, 1:2], 0.0)
            nc.scalar.activation(wdum[:, 5:6], zbias[:], Relu)

            loop_cm = tc.For_i(0, rep, 1) if rep > 1 else contextlib.nullcontext()
            with loop_cm:
             for _inner in range(inner):
              for n in range(n_tiles):
                o = None if "nod2" in ABLATE else ps_o.tile([128, NB], F32, tag="o")
                mids = {}

                # ---- load xT for the whole batch tile in ONE big DMA
                # ([128, 8g, 512b]; small DMAs are fixed-cost dominated)
                if "nodma" not in ABLATE:
                    xtile = xt_pool.tile([128, NG, NB], XDT, tag="xt")
                    nc.sync.dma_start(
                        xtile[:],
                        xt_d[:, :, NB * n:NB * (n + 1)].transpose([1, 0, 2]),
                    )
                else:
                    xtile = xt_fixed

                for g in range(NG):
                    xt = xtile[:, g, :]

                    # ---- dense1: 4 row-tiled concurrent matmuls; u in two
                    # 2-bank halves so drains of one half overlap the next
                    # group's matmuls into the other (pool bufs=3 -> 6 banks)
                    ua = ps_u.tile([128, 2, NB], F32, tag="u")
                    ub = ps_u.tile([128, 2, NB], F32, tag="u")
                    if "nod1" not in ABLATE:
                        for r in range(4):
                            uh = ua if r < 2 else ub
                            nc.tensor.matmul(
                                uh[:, r % 2, :],
                                w1t[32 * r:32 * (r + 1), g, :],
                                xt[32 * r:32 * (r + 1), :],
                                start=True,
                                stop=True,
                                tile_position=(32 * r, 0),
                            )

                    # ---- Schraudolph drain i = int16(A*u + b0), per DPAT
                    iw = iw_pool.tile([128, 4, NB], INT16, tag="I")
                    if "nodrain" not in ABLATE:
                        for hf, uh in ((0, ua), (1, ub)):
                            sl = slice(2 * hf, 2 * hf + 2)
                            on_act = DPAT[g] == "A" or (DPAT[g] == "S" and hf == 0)
                            if on_act:
                                nc.scalar.activation(
                                    iw[:, sl, :], uh[:],
                                    Ident, bias=schra[:, 0:1], scale=SCHRA_A)
                            else:
                                nc.vector.tensor_scalar(
                                    iw[:, sl, :], uh[:],
                                    scalar1=float(SCHRA_A),
                                    scalar2=schra[:, 0:1],
                                    op0=Mult, op1=Add)

                    # ---- T = bitcast(min(i,0) + (16256-c)); R = max(i,0)/A
                    tw = rt_pool.tile([128, 4, NB], INT16, tag="T")
                    rw = rt_pool.tile([128, 4, NB], MID_DT, tag="R")
                    if "notr" not in ABLATE:
                        t_eng = nc.gpsimd if GPST[g] == "G" else nc.vector
                        r_eng = nc.gpsimd if GPSR[g] == "G" else nc.vector
                        t_eng.tensor_scalar(
                            tw[:], iw[:], scalar1=0.0,
                            scalar2=schra[:, 1:2],
                            op0=Min, op1=Add)
                        r_eng.tensor_scalar(
                            rw[:], iw[:], scalar1=0.0,
                            scalar2=float(1.0 / SCHRA_A),
                            op0=Max, op1=Mult)
                    mids[g] = (rw, tw)

                    # ---- dense2 deferred by one group for pipelining
                    if g >= 1 and "nod2" not in ABLATE:
                        _dense2(nc, o, w2t, mids, g - 1)
                if "nod2" not in ABLATE:
                    _dense2(nc, o, w2t, mids, NG - 1, last=True)
                else:
                    mids.clear()

                # ---- drain o PSUM->SBUF and store [p, b] slice
                if "noout" not in ABLATE:
                    outq = outq_pool.tile([128, NB], F32, tag="outq")
                    if OUTQ_ENG == "A":
                        nc.scalar.activation(outq[:], o[:], Copy)
                    else:
                        nc.vector.tensor_copy(outq[:], o[:])
                    nc.sync.dma_start(out_d[:, NB * n:NB * (n + 1)], outq[:])

    return nc


def _dense2(nc, o, w2t, mids, g, last=False):
    """Col-tiled dense2 for group g: 8 matmuls (4 bands x {R, T}) into o."""
    rw, tw = mids[g]
    for c in range(4):
        for si, mid in enumerate((rw, tw)):
            rhs = mid[:, c, :]
            if rhs.dtype == INT16:
                rhs = rhs.bitcast(MID_DT)
            nc.tensor.matmul(
                o[32 * c:32 * (c + 1), :],
                w2t[:, g, c, :],
                rhs,
                start=(g == 0 and si == 0),
                stop=(last and si == 1),
                tile_position=(0, 32 * c),
            )
    del mids[g]


_CACHE = {}


def _get_nc(bc, rep=1, inner=1):
    key = (bc, rep, inner)
    if key not in _CACHE:
        _CACHE[key] = _build(bc, rep, inner)
    return _CACHE[key]


def kernel(x, W1, b1, gamma, beta, mov_mean, mov_var, W2, b2, _rep=1, _inner=1):
    import ml_dtypes

    x = np.asarray(x, np.float32).reshape(-1, C)
    B = x.shape[0]
    w1sb, w2sb, bfin, perm = _host_pack(
        W1, b1, gamma, beta, mov_mean, mov_var, W2, b2
    )

    bc = B // NCORES
    nc = _get_nc(bc, _rep, _inner)

    xT = np.ascontiguousarray(
        x.T.astype(ml_dtypes.bfloat16 if XDT == BF16 else np.float32)
    )  # [C, B]

    schra = np.broadcast_to(
        np.array([SCHRA_B0, SCHRA_BE - SCHRA_C], np.float32), (128, 2)
    ).copy()
    in_maps = [
        {
            "xt": np.ascontiguousarray(xT[:, i * bc:(i + 1) * bc]),
            "w1sb": w1sb,
            "w2sb": w2sb,
            "schra": schra,
        }
        for i in range(NCORES)
    ]
    res = run_bass_kernel_spmd(nc, in_maps, list(range(NCORES)))
    kernel._last_results = res
    # device output is [p, bc] per core; unpermute, transpose + bias on host
    out = np.concatenate(
        [res.results[i]["out"][perm.argsort()].T for i in range(NCORES)], axis=0
    ) + bfin[None, :]
    return np.ascontiguousarray(out, dtype=np.float32)
